# revision 16
# baseline (speedup 1.0000x reference)
"""LoRA multi-head attention on 8 TRN2 NeuronCores.

Sharding: data-parallel over batch (B=8 -> 1 batch element per core),
weights replicated, no collectives. Per-core kernel computes the full
attention block for one (1024, 1024) slice in bf16 with fp32 PSUM
accumulation, in "transposed activation space":

  xT, WqT/WkT/WvT/WoT built via bf16 DRAM-scratch (gpsimd cast DMA)
  + HWDGE DMA-transpose loads.
  qT = (Wq x^T + Bq Aq x^T / 16) / 8      [dout, n]  (1/8 = head scale)
  kT = Wk x^T + Bk Ak x^T / 16            [dout, n]
  v  = x Wv^T + ... (natural [n, dout]), stored per-head with a ones
       column appended ([v_h | 1]) so the PV matmul also produces the
       softmax denominators.
  per head: S^T[m,n] = kT_h^T qT_h ; P^T = exp(S^T) (no max-sub needed,
       |s| is O(4)); O^T/den via [v|1] matmul; normalize with
       reciprocal + ones-outer-product broadcast; assemble attnT.
  out = attnT^T WoT + lora + bo (bias via K=1 ones matmul into PSUM).
"""

import sys

if "/opt/trn_rl_repo" not in sys.path:
    sys.path.insert(0, "/opt/trn_rl_repo")

import numpy as np

N = 1024  # tokens
D = 1024  # model dim
H = 16    # heads
HD = 64   # head dim
R = 16    # lora rank
P = 128   # partitions
F = 512   # psum free-dim tile
NCORES = 8
SCALING = 1.0 / 16.0  # lora alpha/rank
SCALE = HD ** -0.5

_CACHE = {}


def _build():
    import concourse.bacc as bacc
    import concourse.mybir as mybir
    import concourse.tile as tile

    f32 = mybir.dt.float32
    bf16 = mybir.dt.bfloat16
    Exp = mybir.ActivationFunctionType.Exp

    nc = bacc.Bacc("TRN2", target_bir_lowering=False, debug=True)

    x_e = nc.declare_dram_parameter("x", [N, D], f32, isOutput=False)
    w_e = {
        nm: nc.declare_dram_parameter(nm, [D, D], f32, isOutput=False)
        for nm in ("Wq", "Wk", "Wv", "Wo")
    }
    bo_e = nc.declare_dram_parameter("bo", [D], f32, isOutput=False)
    a_e = {
        nm: nc.declare_dram_parameter(nm, [R, D], f32, isOutput=False)
        for nm in ("Aq", "Ak", "Av", "Ao")
    }
    b_e = {
        nm: nc.declare_dram_parameter(nm, [D, R], f32, isOutput=False)
        for nm in ("Bq", "Bk", "Bv", "Bo")
    }
    out_e = nc.declare_dram_parameter("out", [N, D], f32, isOutput=True)

    with tile.TileContext(nc) as tc:
        with (
            tc.tile_pool(name="wpool", bufs=1) as wpool,
            tc.tile_pool(name="dram", bufs=1, space="DRAM") as dram,
            tc.tile_pool(name="stage", bufs=2) as stage,
            tc.tile_pool(name="ps", bufs=1, space="PSUM") as ps,
        ):
            outT = {}
            v_sb = []
            with tc.tile_pool(name="apool", bufs=1) as apool:
                # ---- prep: transposed bf16 copies of x, Wq, Wk, Wv, Wo
                # via contiguous f32 loads + DVE cast + PE transposes
                # (keeps HBM traffic minimal and warms the PE early) ----
                ident = apool.tile([P, P], bf16, tag="ident")
                from concourse.masks import make_identity
                make_identity(nc, ident[:])
                big = {"x": x_e, "Wv": w_e["Wv"], "Wq": w_e["Wq"],
                       "Wk": w_e["Wk"], "Wo": w_e["Wo"]}
                T = {}
                for nm in big:
                    pool = wpool if nm == "Wo" else apool
                    T[nm] = [pool.tile([P, D], bf16, tag=f"T_{nm}_{t}",
                                       name=f"T_{nm}_{t}") for t in range(8)]
                qsel = [nc.sync, nc.scalar]
                for nm, ext in big.items():
                    for t in range(8):
                        st = stage.tile([P, D], f32, tag="ldstage")
                        qsel[t % 2].dma_start(out=st[:],
                                              in_=ext[t * P:(t + 1) * P, :])
                        cb = stage.tile([P, D], bf16, tag="castb")
                        nc.vector.tensor_copy(cb[:], st[:])
                        for c in range(8):
                            pt = ps.tile([P, P], bf16, tag="tpsum", bufs=2)
                            nc.tensor.transpose(
                                pt[:], cb[:, c * P:(c + 1) * P], ident[:])
                            nc.vector.tensor_copy(
                                T[nm][c][:, t * P:(t + 1) * P], pt[:])

                # ---- prep: small lora weights ----
                # A^T via bf16 DRAM scratch + xbar transpose (16-row src ok)
                aT = {}  # A^T: 8 tiles [128, 16] bf16 per name
                for nm, ext in a_e.items():
                    pool = wpool if nm == "Ao" else apool
                    scr = dram.tile([R, D], bf16, tag=f"scr_{nm}")
                    nc.gpsimd.dma_start(out=scr[:], in_=ext[:, :])
                    tiles = []
                    for t in range(8):
                        tt = pool.tile([P, R], bf16, tag=f"aT_{nm}_{t}",
                                       name=f"aT_{nm}_{t}")
                        nc.sync.dma_start(out=tt[:],
                                          in_=scr[:, t * P:(t + 1) * P],
                                          transpose=True)
                        tiles.append(tt)
                    aT[nm] = tiles
                bT = {}  # B^T: [16, 1024] bf16 via PE transpose of chunks
                for nm, ext in b_e.items():
                    pool = wpool if nm == "Bo" else apool
                    scr = dram.tile([D, R], bf16, tag=f"scr_{nm}")
                    nc.gpsimd.dma_start(out=scr[:], in_=ext[:, :])
                    tt = pool.tile([R, D], bf16, tag=f"bT_{nm}",
                                   name=f"bT_{nm}")
                    for t in range(8):
                        bstage = stage.tile([P, R], bf16, tag="bstage")
                        nc.sync.dma_start(out=bstage[:],
                                          in_=scr[t * P:(t + 1) * P, :])
                        pts = ps.tile([R, P], bf16, tag="tpsum", bufs=2)
                        nc.tensor.transpose(pts[:], bstage[:], ident[:])
                        nc.scalar.copy(tt[:, t * P:(t + 1) * P], pts[:])
                    bT[nm] = tt
                # bias + ones helpers
                bo_sb = wpool.tile([1, D], bf16, tag="bo")
                nc.gpsimd.dma_start(out=bo_sb[:], in_=bo_e[None, :])
                ones128 = wpool.tile([1, P], bf16, tag="ones128")
                nc.vector.memset(ones128[:], 1.0)
                onesf = wpool.tile([P, HD], f32, tag="onesf")
                nc.vector.memset(onesf[:], 1.0)

                # ---- phase A0: v natural first (so attention can start
                # as soon as each qT/kT tile pair lands) ----
                tv = apool.tile([R, D], bf16, tag="tvT")
                for nh in range(2):
                    ns = slice(nh * F, (nh + 1) * F)
                    pt = ps.tile([R, F], f32, tag="tpsum", bufs=2)
                    for kt in range(8):
                        nc.tensor.matmul(pt[:], aT["Av"][kt][:],
                                         T["x"][kt][:, ns],
                                         start=(kt == 0), stop=(kt == 7))
                    nc.vector.tensor_scalar_mul(tv[:, ns], pt[:], SCALING)
                VW = H * (HD + 1)  # 1040
                v_sb = [wpool.tile([P, VW], bf16, tag=f"v_{t}",
                                   name=f"v_{t}") for t in range(8)]
                for nt in range(8):
                    vr = v_sb[nt][:].rearrange("p (h c) -> p h c", c=HD + 1)
                    for dh in range(2):
                        ds = slice(dh * F, (dh + 1) * F)
                        pv = ps.tile([P, F], f32, tag="projpsum", bufs=2)
                        for kt in range(8):
                            nc.tensor.matmul(
                                pv[:], T["x"][kt][:, nt * P:(nt + 1) * P],
                                T["Wv"][kt][:, ds],
                                start=(kt == 0), stop=False)
                        nc.tensor.matmul(pv[:], tv[:, nt * P:(nt + 1) * P],
                                         bT["Bv"][:, ds],
                                         start=False, stop=True)
                        pvr = pv[:].rearrange("p (h c) -> p h c", c=HD)
                        nc.vector.tensor_copy(
                            vr[:, dh * 8:(dh + 1) * 8, 0:HD], pvr[:])
                    nc.vector.memset(vr[:, :, HD:HD + 1], 1.0)

                # ---- lora intermediates for q/k (live across dt loop) ----
                tsb = {}
                for nm, anm in (("q", "Aq"), ("k", "Ak")):
                    for nh in range(2):
                        ns = slice(nh * F, (nh + 1) * F)
                        pt = ps.tile([R, F], f32, tag="tpsum", bufs=2)
                        for kt in range(8):
                            nc.tensor.matmul(pt[:], aT[anm][kt][:],
                                             T["x"][kt][:, ns],
                                             start=(kt == 0), stop=(kt == 7))
                        t_s = stage.tile([R, F], bf16, tag="tsb", bufs=4,
                                         name=f"tsb_{nm}_{nh}")
                        nc.vector.tensor_scalar_mul(t_s[:], pt[:], SCALING)
                        tsb[(nm, nh)] = t_s

                # ---- phase A/B interleaved: per dout-tile dt, compute
                # qT[dt], kT[dt], then run attention for heads 2dt, 2dt+1 ----
                outT = {"q": [wpool.tile([P, D], bf16, tag=f"qT_{t}",
                                         name=f"qT_{t}") for t in range(8)],
                        "k": [wpool.tile([P, D], bf16, tag=f"kT_{t}",
                                         name=f"kT_{t}") for t in range(8)]}
                attnT = [wpool.tile([P, D], bf16, tag=f"attnT_{t}",
                                    name=f"attnT_{t}") for t in range(8)]
                for dt in range(8):
                    for nm, wnm, bnm, scl in (("q", "Wq", "Bq", SCALE),
                                              ("k", "Wk", "Bk", None)):
                        dst = outT[nm][dt]
                        for nh in range(2):
                            ns = slice(nh * F, (nh + 1) * F)
                            pq = ps.tile([P, F], f32, tag="projpsum", bufs=2)
                            for kt in range(8):
                                nc.tensor.matmul(
                                    pq[:], T[wnm][kt][:, dt * P:(dt + 1) * P],
                                    T["x"][kt][:, ns],
                                    start=(kt == 0), stop=False)
                            nc.tensor.matmul(pq[:],
                                             bT[bnm][:, dt * P:(dt + 1) * P],
                                             tsb[(nm, nh)][:],
                                             start=False, stop=True)
                            if scl is None:
                                nc.vector.tensor_copy(dst[:, ns], pq[:])
                            else:
                                nc.vector.tensor_scalar_mul(dst[:, ns],
                                                            pq[:], scl)
                    # attention for the two heads living in tile dt.
                    # S matmuls for the head pair are packed onto the PE
                    # array with tile_position: 4 concurrent M=64 matmuls
                    # (2 heads x 2 m-chunks) fill all 16 32x32 subarrays.
                    h0 = 2 * dt
                    qt = outT["q"][dt]
                    ktt = outT["k"][dt]
                    for nh in range(2):
                        ns = slice(nh * F, (nh + 1) * F)
                        po = {}
                        for h in (h0, h0 + 1):
                            po[h] = ps.tile([HD + 1, F], f32, tag="pvpsum",
                                            bufs=2, name=f"po_{h}_{nh}")
                        for mt in range(8):
                            spair = ps.tile([P, 2 * F], f32, tag="spair",
                                            bufs=1)
                            for hi, h in enumerate((h0, h0 + 1)):
                                ro = (h % 2) * HD
                                m0 = mt * P
                                nc.tensor.matmul(
                                    spair[:, hi * F:(hi + 1) * F],
                                    ktt[ro:ro + HD, m0:m0 + P],
                                    qt[ro:ro + HD, ns],
                                    start=True, stop=True)
                            pte = stage.tile([P, 2 * F], bf16, tag="pt",
                                             bufs=2)
                            nc.scalar.activation(pte[:], spair[:], Exp)
                            for hi, h in enumerate((h0, h0 + 1)):
                                nc.tensor.matmul(
                                    po[h][:],
                                    v_sb[mt][:, h * (HD + 1):
                                             (h + 1) * (HD + 1)],
                                    pte[:, hi * F:(hi + 1) * F],
                                    start=(mt == 0), stop=(mt == 7))
                        for h in (h0, h0 + 1):
                            ro = (h % 2) * HD
                            oah = stage.tile([HD + 1, F], f32, tag="oah")
                            nc.vector.tensor_copy(oah[:], po[h][:])
                            pb = ps.tile([HD, F], f32, tag="tpsum", bufs=2)
                            nc.tensor.matmul(pb[:], onesf[HD:HD + 1, :],
                                             oah[HD:HD + 1, :],
                                             start=True, stop=True)
                            pbs = stage.tile([HD, F], f32, tag="pbs")
                            nc.vector.reciprocal_approx_fast(pbs[:], pb[:])
                            ast = stage.tile([HD, F], bf16, tag="ast")
                            nc.vector.tensor_mul(ast[:], oah[0:HD, :],
                                                 pbs[:])
                            nc.sync.dma_start(
                                out=attnT[dt][ro:ro + HD, ns], in_=ast[:])

            # ---- phase C: output projection ----
            to = wpool.tile([R, D], bf16, tag="toT")
            for nh in range(2):
                ns = slice(nh * F, (nh + 1) * F)
                pt = ps.tile([R, F], f32, tag="tpsum", bufs=2)
                for kt in range(8):
                    nc.tensor.matmul(pt[:], aT["Ao"][kt][:],
                                     attnT[kt][:, ns],
                                     start=(kt == 0), stop=(kt == 7))
                nc.scalar.mul(to[:, ns], pt[:], SCALING)
            for nt in range(8):
                for dh in range(2):
                    ds = slice(dh * F, (dh + 1) * F)
                    pf = ps.tile([P, F], f32, tag="projpsum", bufs=2)
                    nc.tensor.matmul(pf[:], ones128[:], bo_sb[:, ds],
                                     start=True, stop=False)
                    for kt in range(8):
                        nc.tensor.matmul(pf[:],
                                         attnT[kt][:, nt * P:(nt + 1) * P],
                                         T["Wo"][kt][:, ds],
                                         start=False, stop=False)
                    nc.tensor.matmul(pf[:], to[:, nt * P:(nt + 1) * P],
                                     bT["Bo"][:, ds], start=False, stop=True)
                    osb = stage.tile([P, F], f32, tag="osb")
                    nc.vector.tensor_copy(osb[:], pf[:])
                    nc.sync.dma_start(out=out_e[nt * P:(nt + 1) * P, ds],
                                      in_=osb[:])
    nc.compile()
    return nc


def _get_nc():
    if "nc" not in _CACHE:
        _CACHE["nc"] = _build()
    return _CACHE["nc"]


def kernel(**inputs):
    from concourse import bass_utils

    nc = _get_nc()
    names = ["Wq", "Wk", "Wv", "Wo", "bo", "Aq", "Bq", "Ak", "Bk",
             "Av", "Bv", "Ao", "Bo"]
    shared = {nm: np.ascontiguousarray(np.asarray(inputs[nm], np.float32))
              for nm in names}
    x = np.ascontiguousarray(np.asarray(inputs["x"], np.float32))
    in_maps = [dict(shared, x=x[i]) for i in range(NCORES)]
    res = bass_utils.run_bass_kernel_spmd(nc, in_maps,
                                          core_ids=list(range(NCORES)))
    return np.stack([res.results[i]["out"] for i in range(NCORES)], axis=0)


# revision 19
# speedup vs baseline: 1.0380x; 1.0380x over previous
"""LoRA multi-head attention on 8 TRN2 NeuronCores.

Sharding: data-parallel over batch (B=8 -> 1 batch element per core),
weights replicated, no collectives. Per-core kernel computes the full
attention block for one (1024, 1024) slice in bf16 with fp32 PSUM
accumulation, in "transposed activation space":

  xT, WqT/WkT/WvT/WoT built via bf16 DRAM-scratch (gpsimd cast DMA)
  + HWDGE DMA-transpose loads.
  qT = (Wq x^T + Bq Aq x^T / 16) / 8      [dout, n]  (1/8 = head scale)
  kT = Wk x^T + Bk Ak x^T / 16            [dout, n]
  v  = x Wv^T + ... (natural [n, dout]), stored per-head with a ones
       column appended ([v_h | 1]) so the PV matmul also produces the
       softmax denominators.
  per head: S^T[m,n] = kT_h^T qT_h ; P^T = exp(S^T) (no max-sub needed,
       |s| is O(4)); O^T/den via [v|1] matmul; normalize with
       reciprocal + ones-outer-product broadcast; assemble attnT.
  out = attnT^T WoT + lora + bo (bias via K=1 ones matmul into PSUM).
"""

import sys

if "/opt/trn_rl_repo" not in sys.path:
    sys.path.insert(0, "/opt/trn_rl_repo")

import numpy as np

N = 1024  # tokens
D = 1024  # model dim
H = 16    # heads
HD = 64   # head dim
R = 16    # lora rank
P = 128   # partitions
F = 512   # psum free-dim tile
NCORES = 8
SCALING = 1.0 / 16.0  # lora alpha/rank
SCALE = HD ** -0.5

_CACHE = {}


def _build():
    import concourse.bacc as bacc
    import concourse.mybir as mybir
    import concourse.tile as tile

    f32 = mybir.dt.float32
    bf16 = mybir.dt.bfloat16
    Exp = mybir.ActivationFunctionType.Exp

    nc = bacc.Bacc("TRN2", target_bir_lowering=False, debug=True)

    x_e = nc.declare_dram_parameter("x", [N, D], f32, isOutput=False)
    w_e = {
        nm: nc.declare_dram_parameter(nm, [D, D], f32, isOutput=False)
        for nm in ("Wq", "Wk", "Wv", "Wo")
    }
    bo_e = nc.declare_dram_parameter("bo", [D], f32, isOutput=False)
    a_e = {
        nm: nc.declare_dram_parameter(nm, [R, D], f32, isOutput=False)
        for nm in ("Aq", "Ak", "Av", "Ao")
    }
    b_e = {
        nm: nc.declare_dram_parameter(nm, [D, R], f32, isOutput=False)
        for nm in ("Bq", "Bk", "Bv", "Bo")
    }
    out_e = nc.declare_dram_parameter("out", [N, D], f32, isOutput=True)

    with tile.TileContext(nc) as tc:
        with (
            tc.tile_pool(name="wpool", bufs=1) as wpool,
            tc.tile_pool(name="dram", bufs=1, space="DRAM") as dram,
            tc.tile_pool(name="stage", bufs=2) as stage,
            tc.tile_pool(name="ps", bufs=1, space="PSUM") as ps,
        ):
            outT = {}
            v_sb = []
            with tc.tile_pool(name="apool", bufs=1) as apool:
                # ---- prep: transposed bf16 copies of x, Wq, Wk, Wv, Wo
                # via contiguous f32 loads + DVE cast + PE transposes
                # (keeps HBM traffic minimal and warms the PE early) ----
                ident = apool.tile([P, P], bf16, tag="ident")
                from concourse.masks import make_identity
                make_identity(nc, ident[:])
                big = {"x": x_e, "Wv": w_e["Wv"], "Wq": w_e["Wq"],
                       "Wk": w_e["Wk"], "Wo": w_e["Wo"]}
                T = {}
                for nm in big:
                    pool = wpool if nm == "Wo" else apool
                    T[nm] = [pool.tile([P, D], bf16, tag=f"T_{nm}_{t}",
                                       name=f"T_{nm}_{t}") for t in range(8)]
                qsel = [nc.sync, nc.scalar]
                # x: PE-transpose path (compute can start within ~15us)
                for t in range(8):
                    st = stage.tile([P, D], f32, tag="ldstage")
                    qsel[t % 2].dma_start(out=st[:],
                                          in_=x_e[t * P:(t + 1) * P, :])
                    cb = stage.tile([P, D], bf16, tag="castb")
                    nc.vector.tensor_copy(cb[:], st[:])
                    for c in range(8):
                        pt = ps.tile([P, P], bf16, tag="tpsum", bufs=2)
                        nc.tensor.transpose(
                            pt[:], cb[:, c * P:(c + 1) * P], ident[:])
                        nc.vector.tensor_copy(
                            T["x"][c][:, t * P:(t + 1) * P], pt[:])
                # weights: load f32 -> DVE cast -> bf16 DRAM scratch ->
                # xbar transpose, spread over both HWDGE queues
                for nm in ("Wv", "Wq", "Wk", "Wo"):
                    ext = big[nm]
                    # two half-tensor scratch tiles -> exact write/read deps
                    scrh = [dram.tile([4 * P, D], bf16, tag=f"scr_{nm}{h}",
                                      name=f"scr_{nm}{h}") for h in range(2)]
                    for t in range(8):
                        st = stage.tile([P, D], f32, tag="ldstage")
                        qsel[t % 2].dma_start(
                            out=st[:], in_=ext[t * P:(t + 1) * P, :])
                        cb = stage.tile([P, D], bf16, tag="castb")
                        nc.vector.tensor_copy(cb[:], st[:])
                        nc.sync.dma_start(
                            out=scrh[t // 4][(t % 4) * P:(t % 4 + 1) * P, :],
                            in_=cb[:])
                    for h in range(2):
                        for c in range(8):
                            nc.sync.dma_start(
                                out=T[nm][c][:, h * 4 * P:(h + 1) * 4 * P],
                                in_=scrh[h][:, c * P:(c + 1) * P],
                                transpose=True)

                # ---- prep: small lora weights ----
                # A^T via bf16 DRAM scratch + xbar transpose (16-row src ok)
                aT = {}  # A^T: 8 tiles [128, 16] bf16 per name
                for nm, ext in a_e.items():
                    pool = wpool if nm == "Ao" else apool
                    scr = dram.tile([R, D], bf16, tag=f"scr_{nm}")
                    nc.gpsimd.dma_start(out=scr[:], in_=ext[:, :])
                    tiles = []
                    for t in range(8):
                        tt = pool.tile([P, R], bf16, tag=f"aT_{nm}_{t}",
                                       name=f"aT_{nm}_{t}")
                        nc.sync.dma_start(out=tt[:],
                                          in_=scr[:, t * P:(t + 1) * P],
                                          transpose=True)
                        tiles.append(tt)
                    aT[nm] = tiles
                bT = {}  # B^T: [16, 1024] bf16 via PE transpose of chunks
                for nm, ext in b_e.items():
                    pool = wpool if nm == "Bo" else apool
                    scr = dram.tile([D, R], bf16, tag=f"scr_{nm}")
                    nc.gpsimd.dma_start(out=scr[:], in_=ext[:, :])
                    tt = pool.tile([R, D], bf16, tag=f"bT_{nm}",
                                   name=f"bT_{nm}")
                    for t in range(8):
                        bstage = stage.tile([P, R], bf16, tag="bstage")
                        nc.sync.dma_start(out=bstage[:],
                                          in_=scr[t * P:(t + 1) * P, :])
                        pts = ps.tile([R, P], bf16, tag="tpsum", bufs=2)
                        nc.tensor.transpose(pts[:], bstage[:], ident[:])
                        nc.scalar.copy(tt[:, t * P:(t + 1) * P], pts[:])
                    bT[nm] = tt
                # bias + ones helpers
                bo_sb = wpool.tile([1, D], bf16, tag="bo")
                nc.gpsimd.dma_start(out=bo_sb[:], in_=bo_e[None, :])
                ones128 = wpool.tile([1, P], bf16, tag="ones128")
                nc.vector.memset(ones128[:], 1.0)
                onesf = wpool.tile([P, HD], f32, tag="onesf")
                nc.vector.memset(onesf[:], 1.0)

                # ---- phase A0: v natural first (so attention can start
                # as soon as each qT/kT tile pair lands) ----
                tv = apool.tile([R, D], bf16, tag="tvT")
                for nh in range(2):
                    ns = slice(nh * F, (nh + 1) * F)
                    pt = ps.tile([R, F], f32, tag="tpsum", bufs=2)
                    for kt in range(8):
                        nc.tensor.matmul(pt[:], aT["Av"][kt][:],
                                         T["x"][kt][:, ns],
                                         start=(kt == 0), stop=(kt == 7))
                    nc.vector.tensor_scalar_mul(tv[:, ns], pt[:], SCALING)
                VW = H * (HD + 1)  # 1040
                v_sb = [wpool.tile([P, VW], bf16, tag=f"v_{t}",
                                   name=f"v_{t}") for t in range(8)]
                for nt in range(8):
                    vr = v_sb[nt][:].rearrange("p (h c) -> p h c", c=HD + 1)
                    for dh in range(2):
                        ds = slice(dh * F, (dh + 1) * F)
                        pv = ps.tile([P, F], f32, tag="projpsum", bufs=2)
                        for kt in range(8):
                            nc.tensor.matmul(
                                pv[:], T["x"][kt][:, nt * P:(nt + 1) * P],
                                T["Wv"][kt][:, ds],
                                start=(kt == 0), stop=False)
                        nc.tensor.matmul(pv[:], tv[:, nt * P:(nt + 1) * P],
                                         bT["Bv"][:, ds],
                                         start=False, stop=True)
                        pvr = pv[:].rearrange("p (h c) -> p h c", c=HD)
                        nc.vector.tensor_copy(
                            vr[:, dh * 8:(dh + 1) * 8, 0:HD], pvr[:])
                    nc.vector.memset(vr[:, :, HD:HD + 1], 1.0)

                # ---- lora intermediates for q/k (live across dt loop) ----
                tsb = {}
                for nm, anm in (("q", "Aq"), ("k", "Ak")):
                    for nh in range(2):
                        ns = slice(nh * F, (nh + 1) * F)
                        pt = ps.tile([R, F], f32, tag="tpsum", bufs=2)
                        for kt in range(8):
                            nc.tensor.matmul(pt[:], aT[anm][kt][:],
                                             T["x"][kt][:, ns],
                                             start=(kt == 0), stop=(kt == 7))
                        t_s = stage.tile([R, F], bf16, tag="tsb", bufs=4,
                                         name=f"tsb_{nm}_{nh}")
                        nc.vector.tensor_scalar_mul(t_s[:], pt[:], SCALING)
                        tsb[(nm, nh)] = t_s

                # ---- phase A/B interleaved: per dout-tile dt, compute
                # qT[dt], kT[dt], then run attention for heads 2dt, 2dt+1 ----
                outT = {"q": [wpool.tile([P, D], bf16, tag=f"qT_{t}",
                                         name=f"qT_{t}") for t in range(8)],
                        "k": [wpool.tile([P, D], bf16, tag=f"kT_{t}",
                                         name=f"kT_{t}") for t in range(8)]}
                attnT = [wpool.tile([P, D], bf16, tag=f"attnT_{t}",
                                    name=f"attnT_{t}") for t in range(8)]
                for dt in range(8):
                    for nm, wnm, bnm, scl in (("q", "Wq", "Bq", SCALE),
                                              ("k", "Wk", "Bk", None)):
                        dst = outT[nm][dt]
                        for nh in range(2):
                            ns = slice(nh * F, (nh + 1) * F)
                            pq = ps.tile([P, F], f32, tag="projpsum", bufs=2)
                            for kt in range(8):
                                nc.tensor.matmul(
                                    pq[:], T[wnm][kt][:, dt * P:(dt + 1) * P],
                                    T["x"][kt][:, ns],
                                    start=(kt == 0), stop=False)
                            nc.tensor.matmul(pq[:],
                                             bT[bnm][:, dt * P:(dt + 1) * P],
                                             tsb[(nm, nh)][:],
                                             start=False, stop=True)
                            if scl is None:
                                nc.vector.tensor_copy(dst[:, ns], pq[:])
                            else:
                                nc.vector.tensor_scalar_mul(dst[:, ns],
                                                            pq[:], scl)
                    # attention for the two heads living in tile dt.
                    # S matmuls for the head pair are packed onto the PE
                    # array with tile_position: 4 concurrent M=64 matmuls
                    # (2 heads x 2 m-chunks) fill all 16 32x32 subarrays.
                    h0 = 2 * dt
                    qt = outT["q"][dt]
                    ktt = outT["k"][dt]
                    for nh in range(2):
                        ns = slice(nh * F, (nh + 1) * F)
                        po = {}
                        for h in (h0, h0 + 1):
                            po[h] = ps.tile([HD + 1, F], f32, tag="pvpsum",
                                            bufs=2, name=f"po_{h}_{nh}")
                        for mt in range(8):
                            spair = ps.tile([P, 2 * F], f32, tag="spair",
                                            bufs=1)
                            for hi, h in enumerate((h0, h0 + 1)):
                                ro = (h % 2) * HD
                                m0 = mt * P
                                nc.tensor.matmul(
                                    spair[:, hi * F:(hi + 1) * F],
                                    ktt[ro:ro + HD, m0:m0 + P],
                                    qt[ro:ro + HD, ns],
                                    start=True, stop=True)
                            pte = stage.tile([P, 2 * F], bf16, tag="pt",
                                             bufs=2)
                            nc.scalar.activation(pte[:], spair[:], Exp)
                            for hi, h in enumerate((h0, h0 + 1)):
                                nc.tensor.matmul(
                                    po[h][:],
                                    v_sb[mt][:, h * (HD + 1):
                                             (h + 1) * (HD + 1)],
                                    pte[:, hi * F:(hi + 1) * F],
                                    start=(mt == 0), stop=(mt == 7))
                        for h in (h0, h0 + 1):
                            ro = (h % 2) * HD
                            oah = stage.tile([HD + 1, F], f32, tag="oah")
                            nc.vector.tensor_copy(oah[:], po[h][:])
                            pb = ps.tile([HD, F], f32, tag="tpsum", bufs=2)
                            nc.tensor.matmul(pb[:], onesf[HD:HD + 1, :],
                                             oah[HD:HD + 1, :],
                                             start=True, stop=True)
                            pbs = stage.tile([HD, F], f32, tag="pbs")
                            nc.vector.reciprocal_approx_fast(pbs[:], pb[:])
                            ast = stage.tile([HD, F], bf16, tag="ast")
                            nc.vector.tensor_mul(ast[:], oah[0:HD, :],
                                                 pbs[:])
                            nc.sync.dma_start(
                                out=attnT[dt][ro:ro + HD, ns], in_=ast[:])

            # ---- phase C: output projection ----
            to = wpool.tile([R, D], bf16, tag="toT")
            for nh in range(2):
                ns = slice(nh * F, (nh + 1) * F)
                pt = ps.tile([R, F], f32, tag="tpsum", bufs=2)
                for kt in range(8):
                    nc.tensor.matmul(pt[:], aT["Ao"][kt][:],
                                     attnT[kt][:, ns],
                                     start=(kt == 0), stop=(kt == 7))
                nc.scalar.mul(to[:, ns], pt[:], SCALING)
            for nt in range(8):
                for dh in range(2):
                    ds = slice(dh * F, (dh + 1) * F)
                    pf = ps.tile([P, F], f32, tag="projpsum", bufs=2)
                    nc.tensor.matmul(pf[:], ones128[:], bo_sb[:, ds],
                                     start=True, stop=False)
                    for kt in range(8):
                        nc.tensor.matmul(pf[:],
                                         attnT[kt][:, nt * P:(nt + 1) * P],
                                         T["Wo"][kt][:, ds],
                                         start=False, stop=False)
                    nc.tensor.matmul(pf[:], to[:, nt * P:(nt + 1) * P],
                                     bT["Bo"][:, ds], start=False, stop=True)
                    osb = stage.tile([P, F], f32, tag="osb")
                    nc.vector.tensor_copy(osb[:], pf[:])
                    nc.sync.dma_start(out=out_e[nt * P:(nt + 1) * P, ds],
                                      in_=osb[:])
    nc.compile()
    return nc


def _get_nc():
    if "nc" not in _CACHE:
        _CACHE["nc"] = _build()
    return _CACHE["nc"]


def kernel(**inputs):
    from concourse import bass_utils

    nc = _get_nc()
    names = ["Wq", "Wk", "Wv", "Wo", "bo", "Aq", "Bq", "Ak", "Bk",
             "Av", "Bv", "Ao", "Bo"]
    shared = {nm: np.ascontiguousarray(np.asarray(inputs[nm], np.float32))
              for nm in names}
    x = np.ascontiguousarray(np.asarray(inputs["x"], np.float32))
    in_maps = [dict(shared, x=x[i]) for i in range(NCORES)]
    res = bass_utils.run_bass_kernel_spmd(nc, in_maps,
                                          core_ids=list(range(NCORES)))
    return np.stack([res.results[i]["out"] for i in range(NCORES)], axis=0)
